# revision 15
# baseline (speedup 1.0000x reference)
"""CNF forward (vector field + exact Jacobian trace) on 8 TRN2 cores.

Math: reference computes, per sample x (row of state[:, 1:]):
    f(x)  = W3^T tanh(W2^T tanh(W1^T [x; t] + b1) + b2) + b3      (dx)
    trJ   = trace(df/dx)                                          (aug = -trJ)

Instead of D=64 JVPs per sample, use the closed form of the trace:
    h1 = tanh([x;t] @ W1 + b1),  h2 = tanh(h1 @ W2 + b2)
    s1 = 1 - h1^2,               s2 = 1 - h2^2
    trJ = s1^T F s2   with  F[h',h] = W2[h',h] * (W3 @ W1[:D])[h, h']
F depends only on the weights and is computed on-device per core
(one K=64 matmul per 128-row tile + an elementwise multiply).

Sharding: data-parallel, 128 samples per core, weights replicated.

Host-side work is layout-only (sharding, zero-FLOP transposes, packing
t/b1 into one bias block); all FLOPs run on device. Layer 1 runs
feature-major (h1T tiles) so W1 itself is the matmul lhsT; layers 2/3
and the trace matmul run batch-major with feature-major activations as
lhsT, giving N=512 fp32 matmuls and no on-device weight transposes.
"""

import numpy as np

import concourse.bacc as bacc
import concourse.bass as bass
import concourse.tile as tile
from concourse import mybir
from concourse.bass_utils import run_bass_kernel_spmd
from concourse.masks import make_identity

B, D, H = 1024, 64, 512
NCORES = 8
BC = B // NCORES  # 128 samples per core
KT = H // 128     # 4 feature tiles of 128
F32 = mybir.dt.float32
AF = mybir.ActivationFunctionType
ALU = mybir.AluOpType
ts = bass.ts

_NC = {}


def _build(with_bias23: bool):
    """with_bias23: include rank-1 bias adds for b2/b3 (batch-major layers
    can't take a per-free-dim bias via ACT). setup_inputs() has zero
    biases so the fast path skips them; nonzero b2/b3 still works."""
    nc = bacc.Bacc()

    stT = nc.declare_dram_parameter("stT", [D, BC], F32, isOutput=False)
    W1x = nc.declare_dram_parameter("W1x", [D, H], F32, isOutput=False)
    W2 = nc.declare_dram_parameter("W2", [H, H], F32, isOutput=False)
    W3 = nc.declare_dram_parameter("W3", [H, D], F32, isOutput=False)
    W3T = nc.declare_dram_parameter("W3T", [D, H], F32, isOutput=False)
    # packed constants: cols 0-3 = b1 + t*W1[D] per feature tile
    cblk = nc.declare_dram_parameter("cblk", [128, KT], F32, isOutput=False)
    if with_bias23:
        b2r = nc.declare_dram_parameter("b2r", [1, H], F32, isOutput=False)
        b3r = nc.declare_dram_parameter("b3r", [1, D], F32, isOutput=False)
    out = nc.declare_dram_parameter("out", [BC, D + 1], F32, isOutput=True)

    with tile.TileContext(nc) as tc:
        with (
            tc.tile_pool(name="const", bufs=1) as cp,
            tc.tile_pool(name="act", bufs=1) as ap,
            tc.tile_pool(name="ps", bufs=1, space="PSUM") as ps,
        ):
            # ------------- loads: critical path first -------------
            # scalar (Act queue): ONLY the small z1-critical loads, so
            # their completion sems aren't delayed behind W2's packets
            stT_sb = ap.tile([D, BC], F32, tag="stT")
            nc.scalar.dma_start(out=stT_sb, in_=stT[:, :])
            w1x = cp.tile([D, H], F32, tag="w1x")
            nc.scalar.dma_start(out=w1x, in_=W1x[:, :])
            cblk_sb = cp.tile([128, KT], F32, tag="cblk")
            nc.scalar.dma_start(out=cblk_sb, in_=cblk[:, :])
            # sync (SP queue): all the fat weights
            w2_sb = []
            for k in range(KT):
                w2k = cp.tile([128, H], F32, tag=f"w2_{k}")
                nc.sync.dma_start(out=w2k, in_=W2[ts(k, 128), :])
                w2_sb.append(w2k)
            w3T_sb = cp.tile([D, H], F32, tag="w3T")
            nc.sync.dma_start(out=w3T_sb, in_=W3T[:, :])
            w3_sb = []
            for k in range(KT):
                w3k = cp.tile([128, D], F32, tag=f"w3_{k}")
                nc.sync.dma_start(out=w3k, in_=W3[ts(k, 128), :])
                w3_sb.append(w3k)
            if with_bias23:
                b2r_sb = cp.tile([1, H], F32, tag="b2r")
                nc.sync.dma_start(out=b2r_sb, in_=b2r[:, :])
                b3r_sb = cp.tile([1, D], F32, tag="b3r")
                nc.sync.dma_start(out=b3r_sb, in_=b3r[:, :])
                onesr = cp.tile([1, BC], F32, tag="onesr")
                nc.vector.memset(onesr, 1.0)
            ident = cp.tile([128, 128], F32, tag="ident")
            make_identity(nc, ident)

            # ------------- layer 1 (feature-major): h1T, s1T -------------
            z1_ps = ps.tile([128, KT * BC], F32, tag="z1", bufs=1)
            h1, s1 = [], []
            for j in range(KT):
                nc.tensor.matmul(z1_ps[:, ts(j, BC)], w1x[:, ts(j, 128)],
                                 stT_sb, start=True, stop=True)
                h = ap.tile([128, BC], F32, tag=f"h1_{j}")
                nc.scalar.activation(h, z1_ps[:, ts(j, BC)], AF.Tanh,
                                     bias=cblk_sb[:, j:j + 1])
                s = ap.tile([128, BC], F32, tag=f"s1_{j}")
                nc.gpsimd.tensor_mul(s, h, h)
                nc.gpsimd.tensor_scalar(s, s, -1.0, 1.0, ALU.mult, ALU.add)
                h1.append(h)
                s1.append(s)

            # ------------- trace weight matrix F -------------
            f_sb = []
            for m in range(KT):
                e2t_ps = ps.tile([128, H], F32, tag="e2t", bufs=2)
                nc.tensor.matmul(e2t_ps, w1x[:, ts(m, 128)], w3T_sb,
                                 start=True, stop=True)
                fm = ap.tile([128, H], F32, tag=f"f_{m}")
                nc.vector.tensor_mul(fm, w2_sb[m], e2t_ps)
                f_sb.append(fm)

            # ------------- layer 2 (batch-major): h2, s2 -------------
            z2_ps = ps.tile([BC, H], F32, tag="z2", bufs=1)
            for k in range(KT):
                nc.tensor.matmul(z2_ps, h1[k], w2_sb[k],
                                 start=(k == 0),
                                 stop=(k == KT - 1 and not with_bias23))
            if with_bias23:
                nc.tensor.matmul(z2_ps, onesr, b2r_sb, start=False, stop=True)
            h2 = ap.tile([BC, H], F32, tag="h2")
            s2 = ap.tile([BC, H], F32, tag="s2")
            for j in range(KT):
                nc.scalar.activation(h2[:, ts(j, 128)], z2_ps[:, ts(j, 128)],
                                     AF.Tanh)
                nc.gpsimd.tensor_mul(s2[:, ts(j, 128)], h2[:, ts(j, 128)],
                                     h2[:, ts(j, 128)])
                nc.gpsimd.tensor_scalar(s2[:, ts(j, 128)], s2[:, ts(j, 128)],
                                        -1.0, 1.0, ALU.mult, ALU.add)

            # ------------- trJ = s1^T F s2 (batch-major) -------------
            t2_ps = ps.tile([BC, H], F32, tag="t2", bufs=1)
            for k in range(KT):
                nc.tensor.matmul(t2_ps, s1[k], f_sb[k],
                                 start=(k == 0), stop=(k == KT - 1))
            final_sb = ap.tile([BC, D + 1], F32, tag="final")
            ttr_scr = ap.tile([BC, H], F32, tag="ttr_scr")
            nc.vector.tensor_mul(ttr_scr, t2_ps, s2)
            nc.vector.tensor_reduce(out=final_sb[:, 0:1], in_=ttr_scr,
                                    op=ALU.add, axis=mybir.AxisListType.X,
                                    negate=True)

            # ------------- layer 3 (batch-major): dx -------------
            h2T_ps = ps.tile([128, KT * BC], F32, tag="h2T", bufs=1)
            for j in range(KT):
                nc.tensor.transpose(h2T_ps[:, ts(j, BC)], h2[:, ts(j, 128)],
                                    ident)
            h2T_sb = ap.tile([128, KT * BC], F32, tag="h2T_sb")
            nc.vector.tensor_copy(h2T_sb[:, 0:2 * BC], h2T_ps[:, 0:2 * BC])
            nc.vector.tensor_copy(h2T_sb[:, 2 * BC:], h2T_ps[:, 2 * BC:])
            o_ps = ps.tile([BC, D], F32, tag="o", bufs=1)
            for k in range(KT):
                nc.tensor.matmul(o_ps, h2T_sb[:, ts(k, BC)], w3_sb[k],
                                 start=(k == 0),
                                 stop=(k == KT - 1 and not with_bias23))
            if with_bias23:
                nc.tensor.matmul(o_ps, onesr, b3r_sb, start=False, stop=True)
            nc.scalar.copy(final_sb[:, 1:D + 1], o_ps)
            nc.sync.dma_start(out=out[:, :], in_=final_sb)

    nc.finalize()
    return nc


def _get_nc(with_bias23: bool):
    key = bool(with_bias23)
    if key not in _NC:
        _NC[key] = _build(key)
    return _NC[key]


def make_in_maps(inputs):
    f32 = lambda a: np.ascontiguousarray(np.asarray(a), dtype=np.float32)
    state = f32(inputs["state"])
    t = float(np.asarray(inputs["t"]).reshape(-1)[0])
    W1 = f32(inputs["W1"])
    b1 = f32(inputs["b1"]).reshape(H)
    W2 = f32(inputs["W2"])
    b2 = f32(inputs["b2"]).reshape(H)
    W3 = f32(inputs["W3"])
    b3 = f32(inputs["b3"]).reshape(D)

    with_bias23 = bool(np.any(b2) or np.any(b3))

    b1_eff = b1 + t * W1[D]                       # fold t-row into bias
    cb = np.ascontiguousarray(b1_eff.reshape(KT, 128).T)

    base = {
        "W1x": np.ascontiguousarray(W1[:D]),
        "W2": W2,
        "W3": W3,
        "W3T": np.ascontiguousarray(W3.T),
        "cblk": cb,
    }
    if with_bias23:
        base["b2r"] = b2.reshape(1, H)
        base["b3r"] = b3.reshape(1, D)
    in_maps = []
    for c in range(NCORES):
        m = dict(base)
        m["stT"] = np.ascontiguousarray(state[c * BC:(c + 1) * BC, 1:].T)
        in_maps.append(m)
    return with_bias23, in_maps


def kernel(**inputs) -> np.ndarray:
    with_bias23, in_maps = make_in_maps(inputs)
    res = run_bass_kernel_spmd(_get_nc(with_bias23), in_maps,
                               list(range(NCORES))).results
    return np.concatenate([res[c]["out"] for c in range(NCORES)], axis=0)


# revision 18
# speedup vs baseline: 1.0616x; 1.0616x over previous
"""CNF forward (vector field + exact Jacobian trace) on 8 TRN2 cores.

Math: reference computes, per sample x (row of state[:, 1:]):
    f(x)  = W3^T tanh(W2^T tanh(W1^T [x; t] + b1) + b2) + b3      (dx)
    trJ   = trace(df/dx)                                          (aug = -trJ)

Instead of D=64 JVPs per sample, use the closed form of the trace:
    h1 = tanh([x;t] @ W1 + b1),  h2 = tanh(h1 @ W2 + b2)
    s1 = 1 - h1^2,               s2 = 1 - h2^2
    trJ = s1^T F s2   with  F[h',h] = W2[h',h] * (W3 @ W1[:D])[h, h']
F depends only on the weights and is computed on-device per core
(one K=64 matmul per 128-row tile + an elementwise multiply).

Sharding: data-parallel, 128 samples per core, weights replicated.

Host-side work is layout-only (sharding, zero-FLOP transposes, packing
t/b1 into one bias block); all FLOPs run on device. Layer 1 runs
feature-major (h1T tiles) so W1 itself is the matmul lhsT; layers 2/3
and the trace matmul run batch-major with feature-major activations as
lhsT, giving N=512 fp32 matmuls and no on-device weight transposes.
"""

import numpy as np

import concourse.bacc as bacc
import concourse.bass as bass
import concourse.tile as tile
from concourse import mybir
from concourse.bass_utils import run_bass_kernel_spmd
from concourse.masks import make_identity
from concourse.tile_rust import add_dep_helper

B, D, H = 1024, 64, 512
NCORES = 8
BC = B // NCORES  # 128 samples per core
KT = H // 128     # 4 feature tiles of 128
F32 = mybir.dt.float32
AF = mybir.ActivationFunctionType
ALU = mybir.AluOpType
ts = bass.ts

_NC = {}

# (engine, tensor) load order; engines: sync=SP HWDGE, scalar=Act HWDGE
DMA_PLAN = [
    ("scalar", "stT"), ("scalar", "w1x"), ("scalar", "cblk"),
    ("sync", "w2_0"), ("sync", "w2_1"), ("sync", "w2_2"), ("sync", "w2_3"),
    ("sync", "w3T"),
    ("sync", "w3_0"), ("sync", "w3_1"), ("sync", "w3_2"), ("sync", "w3_3"),
]


def _build(with_bias23: bool):
    """with_bias23: include rank-1 bias adds for b2/b3 (batch-major layers
    can't take a per-free-dim bias via ACT). setup_inputs() has zero
    biases so the fast path skips them; nonzero b2/b3 still works."""
    nc = bacc.Bacc()

    stT = nc.declare_dram_parameter("stT", [D, BC], F32, isOutput=False)
    W1x = nc.declare_dram_parameter("W1x", [D, H], F32, isOutput=False)
    W2 = nc.declare_dram_parameter("W2", [H, H], F32, isOutput=False)
    W3 = nc.declare_dram_parameter("W3", [H, D], F32, isOutput=False)
    W3T = nc.declare_dram_parameter("W3T", [D, H], F32, isOutput=False)
    # packed constants: cols 0-3 = b1 + t*W1[D] per feature tile
    cblk = nc.declare_dram_parameter("cblk", [128, KT], F32, isOutput=False)
    if with_bias23:
        b2r = nc.declare_dram_parameter("b2r", [1, H], F32, isOutput=False)
        b3r = nc.declare_dram_parameter("b3r", [1, D], F32, isOutput=False)
    out = nc.declare_dram_parameter("out", [BC, D + 1], F32, isOutput=True)

    with tile.TileContext(nc) as tc:
        with (
            tc.tile_pool(name="const", bufs=1) as cp,
            tc.tile_pool(name="act", bufs=1) as ap,
            tc.tile_pool(name="ps", bufs=1, space="PSUM") as ps,
        ):
            # ------------- loads (plan set by DMA_PLAN) -------------
            stT_sb = ap.tile([D, BC], F32, tag="stT")
            w1x = cp.tile([D, H], F32, tag="w1x")
            cblk_sb = cp.tile([128, KT], F32, tag="cblk")
            w2_sb = [cp.tile([128, H], F32, tag=f"w2_{k}", name=f"w2_{k}")
                     for k in range(KT)]
            w3T_sb = cp.tile([D, H], F32, tag="w3T")
            w3_sb = [cp.tile([128, D], F32, tag=f"w3_{k}", name=f"w3_{k}")
                     for k in range(KT)]
            srcs = {"stT": (stT_sb, stT), "w1x": (w1x, W1x),
                    "cblk": (cblk_sb, cblk), "w3T": (w3T_sb, W3T)}
            for k in range(KT):
                srcs[f"w2_{k}"] = (w2_sb[k], W2[ts(k, 128), :])
                srcs[f"w3_{k}"] = (w3_sb[k], W3[ts(k, 128), :])
            for eng, nm in DMA_PLAN:
                dst, src = srcs[nm]
                src = src if isinstance(src, bass.AP) else src[:, :]
                getattr(nc, eng).dma_start(out=dst, in_=src)
            if with_bias23:
                b2r_sb = cp.tile([1, H], F32, tag="b2r")
                nc.sync.dma_start(out=b2r_sb, in_=b2r[:, :])
                b3r_sb = cp.tile([1, D], F32, tag="b3r")
                nc.sync.dma_start(out=b3r_sb, in_=b3r[:, :])
                onesr = cp.tile([1, BC], F32, tag="onesr")
                nc.vector.memset(onesr, 1.0)
            ident = cp.tile([128, 128], F32, tag="ident")
            make_identity(nc, ident)

            # ------------- layer 1 (feature-major): h1T, s1T -------------
            z1_ps = ps.tile([128, KT * BC], F32, tag="z1", bufs=1)
            h1, s1, z1_mm = [], [], []
            for j in range(KT):
                z1_mm.append(
                    nc.tensor.matmul(z1_ps[:, ts(j, BC)], w1x[:, ts(j, 128)],
                                     stT_sb, start=True, stop=True))
                h = ap.tile([128, BC], F32, tag=f"h1_{j}")
                nc.scalar.activation(h, z1_ps[:, ts(j, BC)], AF.Tanh,
                                     bias=cblk_sb[:, j:j + 1])
                s = ap.tile([128, BC], F32, tag=f"s1_{j}")
                nc.gpsimd.tensor_mul(s, h, h)
                nc.gpsimd.tensor_scalar(s, s, -1.0, 1.0, ALU.mult, ALU.add)
                h1.append(h)
                s1.append(s)

            # ------------- layer 2 (batch-major): h2, s2 -------------
            z2_ps = ps.tile([BC, H], F32, tag="z2", bufs=1)
            z2_mm = []
            for k in range(KT):
                z2_mm.append(
                    nc.tensor.matmul(z2_ps, h1[k], w2_sb[k],
                                     start=(k == 0),
                                     stop=(k == KT - 1 and not with_bias23)))
            # PE order: z1 fully before z2 (keeps tanh pipeline tight)
            add_dep_helper(z2_mm[0].ins, z1_mm[KT - 1].ins, sync=False,
                           reason="pe-order z2 after z1")
            if with_bias23:
                nc.tensor.matmul(z2_ps, onesr, b2r_sb, start=False, stop=True)
            h2 = ap.tile([BC, H], F32, tag="h2")
            s2 = ap.tile([BC, H], F32, tag="s2")
            for j in range(KT):
                nc.scalar.activation(h2[:, ts(j, 128)], z2_ps[:, ts(j, 128)],
                                     AF.Tanh)
                nc.gpsimd.tensor_mul(s2[:, ts(j, 128)], h2[:, ts(j, 128)],
                                     h2[:, ts(j, 128)])
                nc.gpsimd.tensor_scalar(s2[:, ts(j, 128)], s2[:, ts(j, 128)],
                                        -1.0, 1.0, ALU.mult, ALU.add)

            # ------------- trace weight matrix F -------------
            f_sb = []
            for m in range(KT):
                e2t_ps = ps.tile([128, H], F32, tag="e2t", bufs=2)
                e2t_mm = nc.tensor.matmul(e2t_ps, w1x[:, ts(m, 128)], w3T_sb,
                                          start=True, stop=True)
                # keep e2t off the DMA window: run after z2 (post-loads)
                add_dep_helper(e2t_mm.ins, z2_mm[KT - 1].ins, sync=False,
                               reason="pe-order e2t after z2")
                fm = ap.tile([128, H], F32, tag=f"f_{m}")
                nc.vector.tensor_mul(fm, w2_sb[m], e2t_ps)
                f_sb.append(fm)

            # ------------- trJ = s1^T F s2 (batch-major) -------------
            t2_ps = ps.tile([BC, H], F32, tag="t2", bufs=1)
            for k in range(KT):
                nc.tensor.matmul(t2_ps, s1[k], f_sb[k],
                                 start=(k == 0), stop=(k == KT - 1))
            final_sb = ap.tile([BC, D + 1], F32, tag="final")
            ttr_scr = ap.tile([BC, H], F32, tag="ttr_scr")
            nc.vector.tensor_mul(ttr_scr, t2_ps, s2)
            nc.vector.tensor_reduce(out=final_sb[:, 0:1], in_=ttr_scr,
                                    op=ALU.add, axis=mybir.AxisListType.X,
                                    negate=True)

            # ------------- layer 3 (batch-major): dx -------------
            h2T_ps = ps.tile([128, KT * BC], F32, tag="h2T", bufs=1)
            for j in range(KT):
                nc.tensor.transpose(h2T_ps[:, ts(j, BC)], h2[:, ts(j, 128)],
                                    ident)
            h2T_sb = ap.tile([128, KT * BC], F32, tag="h2T_sb")
            nc.vector.tensor_copy(h2T_sb[:, 0:2 * BC], h2T_ps[:, 0:2 * BC])
            nc.vector.tensor_copy(h2T_sb[:, 2 * BC:], h2T_ps[:, 2 * BC:])
            o_ps = ps.tile([BC, D], F32, tag="o", bufs=1)
            for k in range(KT):
                nc.tensor.matmul(o_ps, h2T_sb[:, ts(k, BC)], w3_sb[k],
                                 start=(k == 0),
                                 stop=(k == KT - 1 and not with_bias23))
            if with_bias23:
                nc.tensor.matmul(o_ps, onesr, b3r_sb, start=False, stop=True)
            nc.scalar.copy(final_sb[:, 1:D + 1], o_ps)
            nc.sync.dma_start(out=out[:, :], in_=final_sb)

    nc.finalize()
    return nc


def _get_nc(with_bias23: bool):
    key = bool(with_bias23)
    if key not in _NC:
        _NC[key] = _build(key)
    return _NC[key]


def make_in_maps(inputs):
    f32 = lambda a: np.ascontiguousarray(np.asarray(a), dtype=np.float32)
    state = f32(inputs["state"])
    t = float(np.asarray(inputs["t"]).reshape(-1)[0])
    W1 = f32(inputs["W1"])
    b1 = f32(inputs["b1"]).reshape(H)
    W2 = f32(inputs["W2"])
    b2 = f32(inputs["b2"]).reshape(H)
    W3 = f32(inputs["W3"])
    b3 = f32(inputs["b3"]).reshape(D)

    with_bias23 = bool(np.any(b2) or np.any(b3))

    b1_eff = b1 + t * W1[D]                       # fold t-row into bias
    cb = np.ascontiguousarray(b1_eff.reshape(KT, 128).T)

    base = {
        "W1x": np.ascontiguousarray(W1[:D]),
        "W2": W2,
        "W3": W3,
        "W3T": np.ascontiguousarray(W3.T),
        "cblk": cb,
    }
    if with_bias23:
        base["b2r"] = b2.reshape(1, H)
        base["b3r"] = b3.reshape(1, D)
    in_maps = []
    for c in range(NCORES):
        m = dict(base)
        m["stT"] = np.ascontiguousarray(state[c * BC:(c + 1) * BC, 1:].T)
        in_maps.append(m)
    return with_bias23, in_maps


def kernel(**inputs) -> np.ndarray:
    with_bias23, in_maps = make_in_maps(inputs)
    res = run_bass_kernel_spmd(_get_nc(with_bias23), in_maps,
                               list(range(NCORES))).results
    return np.concatenate([res[c]["out"] for c in range(NCORES)], axis=0)


# revision 19
# speedup vs baseline: 1.1186x; 1.0538x over previous
"""CNF forward (vector field + exact Jacobian trace) on 8 TRN2 cores.

Math: reference computes, per sample x (row of state[:, 1:]):
    f(x)  = W3^T tanh(W2^T tanh(W1^T [x; t] + b1) + b2) + b3      (dx)
    trJ   = trace(df/dx)                                          (aug = -trJ)

Instead of D=64 JVPs per sample, use the closed form of the trace:
    h1 = tanh([x;t] @ W1 + b1),  h2 = tanh(h1 @ W2 + b2)
    s1 = 1 - h1^2,               s2 = 1 - h2^2
    trJ = s1^T F s2   with  F[h',h] = W2[h',h] * (W3 @ W1[:D])[h, h']
F depends only on the weights and is computed on-device per core
(one K=64 matmul per 128-row tile + an elementwise multiply).

Sharding: data-parallel, 128 samples per core, weights replicated.

Host-side work is layout-only (sharding, zero-FLOP transposes, packing
t/b1 into one bias block); all FLOPs run on device. Layer 1 runs
feature-major (h1T tiles) so W1 itself is the matmul lhsT; layers 2/3
and the trace matmul run batch-major with feature-major activations as
lhsT, giving N=512 fp32 matmuls and no on-device weight transposes.
"""

import numpy as np

import concourse.bacc as bacc
import concourse.bass as bass
import concourse.tile as tile
from concourse import mybir
from concourse.bass_utils import run_bass_kernel_spmd
from concourse.masks import make_identity
from concourse.tile_rust import add_dep_helper

B, D, H = 1024, 64, 512
NCORES = 8
BC = B // NCORES  # 128 samples per core
KT = H // 128     # 4 feature tiles of 128
F32 = mybir.dt.float32
AF = mybir.ActivationFunctionType
ALU = mybir.AluOpType
ts = bass.ts

_NC = {}

# (engine, tensor) load order; engines: sync=SP HWDGE, scalar=Act HWDGE
DMA_PLAN = [
    ("scalar", "stT"), ("scalar", "w1x"), ("scalar", "cblk"),
    ("sync", "w2_0"), ("sync", "w2_1"), ("sync", "w2_2"), ("sync", "w2_3"),
    ("sync", "w3T"),
    ("sync", "w3_0"), ("sync", "w3_1"), ("sync", "w3_2"), ("sync", "w3_3"),
]


def _build(with_bias23: bool):
    """with_bias23: include rank-1 bias adds for b2/b3 (batch-major layers
    can't take a per-free-dim bias via ACT). setup_inputs() has zero
    biases so the fast path skips them; nonzero b2/b3 still works."""
    nc = bacc.Bacc()

    stT = nc.declare_dram_parameter("stT", [D, BC], F32, isOutput=False)
    W1x = nc.declare_dram_parameter("W1x", [D, H], F32, isOutput=False)
    W2 = nc.declare_dram_parameter("W2", [H, H], F32, isOutput=False)
    W3 = nc.declare_dram_parameter("W3", [H, D], F32, isOutput=False)
    W3T = nc.declare_dram_parameter("W3T", [D, H], F32, isOutput=False)
    # packed constants: cols 0-3 = b1 + t*W1[D] per feature tile
    cblk = nc.declare_dram_parameter("cblk", [128, KT], F32, isOutput=False)
    if with_bias23:
        b2r = nc.declare_dram_parameter("b2r", [1, H], F32, isOutput=False)
        b3r = nc.declare_dram_parameter("b3r", [1, D], F32, isOutput=False)
    out = nc.declare_dram_parameter("out", [BC, D + 1], F32, isOutput=True)

    with tile.TileContext(nc) as tc:
        with (
            tc.tile_pool(name="const", bufs=1) as cp,
            tc.tile_pool(name="act", bufs=1) as ap,
            tc.tile_pool(name="ps", bufs=1, space="PSUM") as ps,
        ):
            # ------------- loads (plan set by DMA_PLAN) -------------
            stT_sb = ap.tile([D, BC], F32, tag="stT")
            w1x = cp.tile([D, H], F32, tag="w1x")
            cblk_sb = cp.tile([128, KT], F32, tag="cblk")
            w2_sb = [cp.tile([128, H], F32, tag=f"w2_{k}", name=f"w2_{k}")
                     for k in range(KT)]
            w3T_sb = cp.tile([D, H], F32, tag="w3T")
            w3_sb = [cp.tile([128, D], F32, tag=f"w3_{k}", name=f"w3_{k}")
                     for k in range(KT)]
            srcs = {"stT": (stT_sb, stT), "w1x": (w1x, W1x),
                    "cblk": (cblk_sb, cblk), "w3T": (w3T_sb, W3T)}
            for k in range(KT):
                srcs[f"w2_{k}"] = (w2_sb[k], W2[ts(k, 128), :])
                srcs[f"w3_{k}"] = (w3_sb[k], W3[ts(k, 128), :])
            for eng, nm in DMA_PLAN:
                dst, src = srcs[nm]
                src = src if isinstance(src, bass.AP) else src[:, :]
                getattr(nc, eng).dma_start(out=dst, in_=src)
            if with_bias23:
                b2r_sb = cp.tile([1, H], F32, tag="b2r")
                nc.sync.dma_start(out=b2r_sb, in_=b2r[:, :])
                b3r_sb = cp.tile([1, D], F32, tag="b3r")
                nc.sync.dma_start(out=b3r_sb, in_=b3r[:, :])
                onesr = cp.tile([1, BC], F32, tag="onesr")
                nc.vector.memset(onesr, 1.0)
            ident = cp.tile([128, 128], F32, tag="ident")
            make_identity(nc, ident)

            # ------------- layer 1 (feature-major): h1T, s1T -------------
            h1, s1, z1_mm = [], [], []
            for j in range(KT):
                z1_ps = ps.tile([128, BC], F32, tag="z1", bufs=2)
                z1_mm.append(
                    nc.tensor.matmul(z1_ps, w1x[:, ts(j, 128)],
                                     stT_sb, start=True, stop=True))
                h = ap.tile([128, BC], F32, tag=f"h1_{j}")
                nc.scalar.activation(h, z1_ps, AF.Tanh,
                                     bias=cblk_sb[:, j:j + 1])
                s = ap.tile([128, BC], F32, tag=f"s1_{j}")
                nc.gpsimd.tensor_mul(s, h, h)
                nc.gpsimd.tensor_scalar(s, s, -1.0, 1.0, ALU.mult, ALU.add)
                h1.append(h)
                s1.append(s)

            # ------------- layer 2 (batch-major): h2, s2 -------------
            z2_ps = ps.tile([BC, H], F32, tag="z2", bufs=1)
            z2_mm = []
            for k in range(KT):
                z2_mm.append(
                    nc.tensor.matmul(z2_ps, h1[k], w2_sb[k],
                                     start=(k == 0),
                                     stop=(k == KT - 1 and not with_bias23)))
            # PE order: z1 fully before z2 (keeps tanh pipeline tight)
            add_dep_helper(z2_mm[0].ins, z1_mm[KT - 1].ins, sync=False,
                           reason="pe-order z2 after z1")
            if with_bias23:
                nc.tensor.matmul(z2_ps, onesr, b2r_sb, start=False, stop=True)
            h2 = ap.tile([BC, H], F32, tag="h2")
            s2 = ap.tile([BC, H], F32, tag="s2")
            for j in range(KT):
                nc.scalar.activation(h2[:, ts(j, 128)], z2_ps[:, ts(j, 128)],
                                     AF.Tanh)
                nc.gpsimd.tensor_mul(s2[:, ts(j, 128)], h2[:, ts(j, 128)],
                                     h2[:, ts(j, 128)])
                nc.gpsimd.tensor_scalar(s2[:, ts(j, 128)], s2[:, ts(j, 128)],
                                        -1.0, 1.0, ALU.mult, ALU.add)

            # ------------- trace weight matrix F -------------
            f_sb = []
            for m in range(KT):
                e2t_ps = ps.tile([128, H], F32, tag="e2t", bufs=2)
                e2t_mm = nc.tensor.matmul(e2t_ps, w1x[:, ts(m, 128)], w3T_sb,
                                          start=True, stop=True)
                # keep e2t off the DMA window: run after z2 (post-loads)
                add_dep_helper(e2t_mm.ins, z2_mm[KT - 1].ins, sync=False,
                               reason="pe-order e2t after z2")
                fm = ap.tile([128, H], F32, tag=f"f_{m}")
                nc.vector.tensor_mul(fm, w2_sb[m], e2t_ps)
                f_sb.append(fm)

            # ------------- trJ = s1^T F s2 (batch-major) -------------
            t2_ps = ps.tile([BC, H], F32, tag="t2", bufs=1)
            for k in range(KT):
                nc.tensor.matmul(t2_ps, s1[k], f_sb[k],
                                 start=(k == 0), stop=(k == KT - 1))
            final_sb = ap.tile([BC, D + 1], F32, tag="final")
            ttr_scr = ap.tile([BC, H], F32, tag="ttr_scr")
            nc.vector.tensor_mul(ttr_scr, t2_ps, s2)
            nc.vector.tensor_reduce(out=final_sb[:, 0:1], in_=ttr_scr,
                                    op=ALU.add, axis=mybir.AxisListType.X,
                                    negate=True)

            # ------------- layer 3 (batch-major): dx -------------
            h2T_ps = ps.tile([128, KT * BC], F32, tag="h2T", bufs=1)
            for j in range(KT):
                nc.tensor.transpose(h2T_ps[:, ts(j, BC)], h2[:, ts(j, 128)],
                                    ident)
            h2T_sb = ap.tile([128, KT * BC], F32, tag="h2T_sb")
            nc.vector.tensor_copy(h2T_sb[:, 0:2 * BC], h2T_ps[:, 0:2 * BC])
            nc.vector.tensor_copy(h2T_sb[:, 2 * BC:], h2T_ps[:, 2 * BC:])
            o_ps = ps.tile([BC, D], F32, tag="o", bufs=1)
            for k in range(KT):
                nc.tensor.matmul(o_ps, h2T_sb[:, ts(k, BC)], w3_sb[k],
                                 start=(k == 0),
                                 stop=(k == KT - 1 and not with_bias23))
            if with_bias23:
                nc.tensor.matmul(o_ps, onesr, b3r_sb, start=False, stop=True)
            nc.scalar.copy(final_sb[:, 1:D + 1], o_ps)
            nc.sync.dma_start(out=out[:, :], in_=final_sb)

    nc.finalize()
    return nc


def _get_nc(with_bias23: bool):
    key = bool(with_bias23)
    if key not in _NC:
        _NC[key] = _build(key)
    return _NC[key]


def make_in_maps(inputs):
    f32 = lambda a: np.ascontiguousarray(np.asarray(a), dtype=np.float32)
    state = f32(inputs["state"])
    t = float(np.asarray(inputs["t"]).reshape(-1)[0])
    W1 = f32(inputs["W1"])
    b1 = f32(inputs["b1"]).reshape(H)
    W2 = f32(inputs["W2"])
    b2 = f32(inputs["b2"]).reshape(H)
    W3 = f32(inputs["W3"])
    b3 = f32(inputs["b3"]).reshape(D)

    with_bias23 = bool(np.any(b2) or np.any(b3))

    b1_eff = b1 + t * W1[D]                       # fold t-row into bias
    cb = np.ascontiguousarray(b1_eff.reshape(KT, 128).T)

    base = {
        "W1x": np.ascontiguousarray(W1[:D]),
        "W2": W2,
        "W3": W3,
        "W3T": np.ascontiguousarray(W3.T),
        "cblk": cb,
    }
    if with_bias23:
        base["b2r"] = b2.reshape(1, H)
        base["b3r"] = b3.reshape(1, D)
    in_maps = []
    for c in range(NCORES):
        m = dict(base)
        m["stT"] = np.ascontiguousarray(state[c * BC:(c + 1) * BC, 1:].T)
        in_maps.append(m)
    return with_bias23, in_maps


def kernel(**inputs) -> np.ndarray:
    with_bias23, in_maps = make_in_maps(inputs)
    res = run_bass_kernel_spmd(_get_nc(with_bias23), in_maps,
                               list(range(NCORES))).results
    return np.concatenate([res[c]["out"] for c in range(NCORES)], axis=0)
